# revision 12
# baseline (speedup 1.0000x reference)
"""LipschitzRNN Trainium2 kernel.

Math (per reference):
    bA = 0.5*exp(-bA_z^2)+0.5 ; bW likewise
    A = (1-bA)(MA+MA.T) + bA(MA-MA.T) - YA*I
    C = (1-bA)(MW+MW.T) + bW(MW-MW.T) - YW*I
    X_{t+1} = X_t + STEP*(A@X_t + tanh(C@X_t + by))   (column-state X: [n, bs])
    out[b, t, :] = X_t[:, b]

Device strategy (8-way batch data-parallel, b=32/core):
  - State kept as [n(partitions), b]; fp16 PE operands, fp32 master state so
    fp16 error only enters through STEP-scaled paths (unfolded update).
  - Critical-path shortening via linearity:
        P_i  = X_{i-1} + STEP*U_i,   X_i = P_i + STEP*T_i
        V_{i+1} = C@X_i = C@P_i + (STEP*C)@T_i
    so the recurrence chain is tanh_i -> 4 small matmuls -> tanh_{i+1};
    the bias `by` is injected into the V PSUM group with a rank-1 matmul.
  - U path uses xq = fp16(X); master ops on VectorE, fp16 casts on GpSimd.
  - Output rows via PE transpose of xq, batched stage copy + DMA.
"""

import os
import numpy as np

N = 256
BS = 256
TMAX = 512
STEP = 0.01
YA = 0.001
YW = 0.001
NCORES = 8
BLOC = BS // NCORES  # 32

LAST_RESULT = None  # BassKernelResults of the most recent run (for test harness)


def _build(n_steps):
    from concourse import bacc, tile
    import concourse.mybir as mybir
    from concourse.masks import make_identity

    F32 = mybir.dt.float32
    F16 = mybir.dt.float16
    AF = mybir.ActivationFunctionType
    ALU = mybir.AluOpType

    nc = bacc.Bacc("TRN2", target_bir_lowering=False, debug=False,
                   num_devices=NCORES)

    WA = nc.dram_tensor("WA", [N, N], F16, kind="ExternalInput")    # A.T      [k, m]
    WC = nc.dram_tensor("WC", [N, N], F16, kind="ExternalInput")    # C.T      [k, m]
    WCS = nc.dram_tensor("WCS", [N, N], F16, kind="ExternalInput")  # (STEP*C).T
    BYR = nc.dram_tensor("BYR", [1, N], F16, kind="ExternalInput")  # by as one row
    ONES = nc.dram_tensor("ONES", [1, BLOC], F16, kind="ExternalInput")
    X0T = nc.dram_tensor("X0T", [N, BLOC], F32, kind="ExternalInput")
    OUT = nc.dram_tensor("OUT", [BLOC, TMAX, N], F32, kind="ExternalOutput")

    with tile.TileContext(nc) as tc:
        with (
            tc.tile_pool(name="consts", bufs=1) as consts,
            tc.tile_pool(name="xqpool", bufs=3) as xqpool,
            tc.tile_pool(name="xpool", bufs=3) as xpool,
            tc.tile_pool(name="ppool", bufs=3) as ppool,
            tc.tile_pool(name="pqpool", bufs=3) as pqpool,
            tc.tile_pool(name="tpool", bufs=3) as tpool,
            tc.tile_pool(name="stpool", bufs=4) as stpool,
            tc.tile_pool(name="psv", bufs=2, space="PSUM") as psv,
            tc.tile_pool(name="psu", bufs=2, space="PSUM") as psu,
            tc.tile_pool(name="pst", bufs=3, space="PSUM") as pst,
        ):
            # ---- constants ----
            wa = [[consts.tile([128, 128], F16, name=f"wa{k}{mc}", tag=f"wa{k}{mc}")
                   for mc in range(2)] for k in range(2)]
            wc = [[consts.tile([128, 128], F16, name=f"wc{k}{mc}", tag=f"wc{k}{mc}")
                   for mc in range(2)] for k in range(2)]
            wcs = [[consts.tile([128, 128], F16, name=f"wcs{k}{mc}", tag=f"wcs{k}{mc}")
                    for mc in range(2)] for k in range(2)]
            for k in range(2):
                for mc in range(2):
                    nc.sync.dma_start(wa[k][mc][:], WA[128 * k:128 * (k + 1), 128 * mc:128 * (mc + 1)])
                    nc.sync.dma_start(wc[k][mc][:], WC[128 * k:128 * (k + 1), 128 * mc:128 * (mc + 1)])
                    nc.sync.dma_start(wcs[k][mc][:], WCS[128 * k:128 * (k + 1), 128 * mc:128 * (mc + 1)])
            byr = consts.tile([1, N], F16, tag="byr")
            nc.sync.dma_start(byr[:], BYR[:])
            ones = consts.tile([1, BLOC], F16, tag="ones")
            nc.sync.dma_start(ones[:], ONES[:])
            ident_f32 = consts.tile([128, 128], F32, tag="ident_f32")
            make_identity(nc, ident_f32[:])
            ident = consts.tile([128, 128], F16, tag="ident")
            nc.vector.tensor_copy(ident[:], ident_f32[:])

            # ---- initial state ----
            x = xpool.tile([128, 2 * BLOC], F32, tag="x")    # master fp32 X
            nc.sync.dma_start(x[:, 0:BLOC], X0T[0:128, :])
            nc.sync.dma_start(x[:, BLOC:2 * BLOC], X0T[128:256, :])
            xq = xqpool.tile([128, 2 * BLOC], F16, tag="xq")
            nc.vector.tensor_copy(xq[:], x[:])

            # bootstrap pv_1 = C@xq0 + by
            pv_cur = psv.tile([128, 2 * BLOC], F32, tag="pv")
            for mc in range(2):
                ms = slice(BLOC * mc, BLOC * (mc + 1))
                nc.tensor.matmul(pv_cur[:, ms], byr[0:1, 128 * mc:128 * (mc + 1)],
                                 ones[:], start=True, stop=False)
                nc.tensor.matmul(pv_cur[:, ms], wc[0][mc][:], xq[:, 0:BLOC],
                                 start=False, stop=False)
                nc.tensor.matmul(pv_cur[:, ms], wc[1][mc][:], xq[:, BLOC:2 * BLOC],
                                 start=False, stop=True)

            # ---- recurrence ----
            GRP = 4  # output steps batched per stage copy/DMA
            pt = None
            for t in range(1, n_steps + 1):
                g = (t - 1) % GRP
                gn = min(GRP, n_steps - (t - 1 - g))  # size of this group
                last = (t == n_steps)

                # U_t = A@xq_{t-1} (early; input from previous iteration)
                pu = psu.tile([128, 2 * BLOC], F32, tag="pu")
                for mc in range(2):
                    ms = slice(BLOC * mc, BLOC * (mc + 1))
                    nc.tensor.matmul(pu[:, ms], wa[0][mc][:], xq[:, 0:BLOC],
                                     start=True, stop=False)
                    nc.tensor.matmul(pu[:, ms], wa[1][mc][:], xq[:, BLOC:2 * BLOC],
                                     start=False, stop=True)

                if not last:
                    # bias injection for pv_next (independent; runs early)
                    pv_next = psv.tile([128, 2 * BLOC], F32, tag="pv")
                    for mc in range(2):
                        ms = slice(BLOC * mc, BLOC * (mc + 1))
                        nc.tensor.matmul(pv_next[:, ms],
                                         byr[0:1, 128 * mc:128 * (mc + 1)],
                                         ones[:], start=True, stop=False)

                # T_t = tanh(V_t) fp16, single op (bias already inside pv_cur)
                tt = tpool.tile([128, 2 * BLOC], F16, tag="tt")
                nc.scalar.activation(tt[:], pv_cur[:], AF.Tanh, scale=1.0)

                # P_t = X_{t-1} + STEP*U_t   (during tanh)
                p = ppool.tile([128, 2 * BLOC], F32, tag="p")
                nc.vector.scalar_tensor_tensor(
                    p[:], pu[:], STEP, x[:], op0=ALU.mult, op1=ALU.add)
                pq = pqpool.tile([128, 2 * BLOC], F16, tag="pq")
                nc.gpsimd.tensor_copy(pq[:], p[:])

                if not last:
                    # V_{t+1} += C@P_t  (off chain)
                    for mc in range(2):
                        ms = slice(BLOC * mc, BLOC * (mc + 1))
                        nc.tensor.matmul(pv_next[:, ms], wc[0][mc][:], pq[:, 0:BLOC],
                                         start=False, stop=False)
                        nc.tensor.matmul(pv_next[:, ms], wc[1][mc][:],
                                         pq[:, BLOC:2 * BLOC], start=False, stop=False)

                # X_t = P_t + STEP*T_t   (master, off chain)
                x = xpool.tile([128, 2 * BLOC], F32, tag="x")
                nc.vector.scalar_tensor_tensor(
                    x[:], tt[:], STEP, p[:], op0=ALU.mult, op1=ALU.add)
                xq = xqpool.tile([128, 2 * BLOC], F16, tag="xq")
                nc.gpsimd.tensor_copy(xq[:], x[:])

                if not last:
                    # V_{t+1} += (STEP*C)@T_t  -- THE chain: tanh -> these -> tanh
                    for mc in range(2):
                        ms = slice(BLOC * mc, BLOC * (mc + 1))
                        nc.tensor.matmul(pv_next[:, ms], wcs[0][mc][:], tt[:, 0:BLOC],
                                         start=False, stop=False)
                        nc.tensor.matmul(pv_next[:, ms], wcs[1][mc][:],
                                         tt[:, BLOC:2 * BLOC], start=False, stop=True)
                    pv_cur = pv_next

                # output row t: transpose xq_t into GRP-step PSUM batch
                if g == 0:
                    pt = pst.tile([BLOC, GRP, N], F16, tag="pt")
                nc.tensor.transpose(pt[:, g, 0:128], xq[:, 0:BLOC], ident[:])
                nc.tensor.transpose(pt[:, g, 128:256], xq[:, BLOC:2 * BLOC], ident[:])
                if g == gn - 1:
                    stage = stpool.tile([BLOC, GRP, N], F32, tag="stage")
                    half = (gn + 1) // 2
                    nc.vector.tensor_copy(stage[:, 0:half], pt[:, 0:half])
                    nc.scalar.copy(stage[:, half:gn], pt[:, half:gn])
                    nc.sync.dma_start(OUT[:, t - gn + 1:t + 1, :], stage[:, 0:gn])
    nc.compile()
    return nc


def kernel(X0, MA, MW, bA_z, bW_z, by_w):
    global LAST_RESULT
    from concourse.bass_utils import run_bass_kernel_spmd

    X0 = np.asarray(X0, dtype=np.float32)
    MA = np.asarray(MA, dtype=np.float32)
    MW = np.asarray(MW, dtype=np.float32)
    bA_z = np.asarray(bA_z, dtype=np.float32)
    bW_z = np.asarray(bW_z, dtype=np.float32)
    by_w = np.asarray(by_w, dtype=np.float32)

    # host-side weight prep (f32, matches reference math); fp16 PE operands
    bA = np.float32(0.5) * np.exp(-bA_z[0, 0] * bA_z[0, 0]) + np.float32(0.5)
    bW = np.float32(0.5) * np.exp(-bW_z[0, 0] * bW_z[0, 0]) + np.float32(0.5)
    I = np.eye(N, dtype=np.float32)
    A = (1 - bA) * (MA + MA.T) + bA * (MA - MA.T) - np.float32(YA) * I
    C = (1 - bA) * (MW + MW.T) + bW * (MW - MW.T) - np.float32(YW) * I
    WAa = np.ascontiguousarray(A.T).astype(np.float16)
    WCa = np.ascontiguousarray(C.T).astype(np.float16)
    WCSa = np.ascontiguousarray((np.float32(STEP) * C).T).astype(np.float16)
    BYRa = np.ascontiguousarray(by_w.reshape(1, N)).astype(np.float16)
    ONESa = np.ones((1, BLOC), dtype=np.float16)

    n_steps = TMAX - 1
    in_maps = []
    for i in range(NCORES):
        in_maps.append({
            "WA": WAa,
            "WC": WCa,
            "WCS": WCSa,
            "BYR": BYRa,
            "ONES": ONESa,
            "X0T": np.ascontiguousarray(X0[i * BLOC:(i + 1) * BLOC, :].T),
        })

    nc = _build(n_steps)
    res = run_bass_kernel_spmd(nc, in_maps, core_ids=list(range(NCORES)))
    LAST_RESULT = res

    out = np.concatenate([r["OUT"] for r in res.results], axis=0)
    out[:, 0, :] = X0
    return out


if __name__ == "__main__":
    rng = np.random.default_rng(0)
    inputs = {
        "X0": rng.standard_normal((BS, N), dtype=np.float32),
        "MA": rng.standard_normal((N, N), dtype=np.float32) / 16,
        "MW": rng.standard_normal((N, N), dtype=np.float32) / 16,
        "bA_z": np.full((1, 1), 0.65, dtype=np.float32),
        "bW_z": np.full((1, 1), 0.65, dtype=np.float32),
        "by_w": rng.standard_normal((N, 1), dtype=np.float32) / 100,
    }
    out = kernel(**inputs)
    print("out", out.shape, out.dtype, np.abs(out).max())


# revision 13
# speedup vs baseline: 1.5404x; 1.5404x over previous
"""LipschitzRNN Trainium2 kernel.

Math (per reference):
    bA = 0.5*exp(-bA_z^2)+0.5 ; bW likewise
    A = (1-bA)(MA+MA.T) + bA(MA-MA.T) - YA*I
    C = (1-bA)(MW+MW.T) + bW(MW-MW.T) - YW*I
    X_{t+1} = X_t + STEP*(A@X_t + tanh(C@X_t + by))   (column-state X: [n, bs])
    out[b, t, :] = X_t[:, b]

Device strategy (8-way batch data-parallel, b=32/core):
  - State kept as [n(partitions), b] in SBUF: two k-chunks side by side [128, 64].
  - Weights stationary per matmul ([A-fold; C] in lhsT [k, m] layout), float32r
    (fp22 multiplies, fp32 accumulate) self-loading matmuls.
  - Folded form: G = (I + STEP*A)@X via WG, V = C@X via WC;
    tanh fused with per-partition bias `by` on ScalarE;
    X_next = (tanh * STEP) + G in one VectorE scalar_tensor_tensor.
  - Output needs [b, n] rows: PE transpose each new state, DVE copy to SBUF,
    DMA to OUT[b, t, :].
"""

import os
import numpy as np

N = 256
BS = 256
TMAX = 512
STEP = 0.01
YA = 0.001
YW = 0.001
NCORES = 8
BLOC = BS // NCORES  # 32

LAST_RESULT = None  # BassKernelResults of the most recent run (for test harness)


def _build(n_steps):
    from concourse import bacc, tile
    import concourse.mybir as mybir
    from concourse.masks import make_identity

    F32 = mybir.dt.float32
    F16 = mybir.dt.float16
    AF = mybir.ActivationFunctionType
    ALU = mybir.AluOpType

    nc = bacc.Bacc("TRN2", target_bir_lowering=False, debug=False,
                   num_devices=NCORES)

    WA = nc.dram_tensor("WA", [N, N], F16, kind="ExternalInput")    # A.T  [k, m]
    WC = nc.dram_tensor("WC", [N, N], F16, kind="ExternalInput")    # C.T  [k, m]
    BY = nc.dram_tensor("BY", [N, 1], F32, kind="ExternalInput")
    X0T = nc.dram_tensor("X0T", [N, BLOC], F32, kind="ExternalInput")
    OUT = nc.dram_tensor("OUT", [BLOC, TMAX, N], F32, kind="ExternalOutput")

    with tile.TileContext(nc) as tc:
        with (
            tc.tile_pool(name="consts", bufs=1) as consts,
            tc.tile_pool(name="xqpool", bufs=3) as xqpool,
            tc.tile_pool(name="mpool", bufs=3) as mpool,
            tc.tile_pool(name="ppool", bufs=2) as ppool,
            tc.tile_pool(name="tpool", bufs=2) as tpool,
            tc.tile_pool(name="stpool", bufs=4) as stpool,
            tc.tile_pool(name="psv", bufs=2, space="PSUM") as psv,
            tc.tile_pool(name="psu", bufs=2, space="PSUM") as psu,
            tc.tile_pool(name="pst", bufs=3, space="PSUM") as pst,
        ):
            # ---- constants / initial state ----
            wa = [[consts.tile([128, 128], F16, name=f"wa{k}{mc}", tag=f"wa{k}{mc}")
                   for mc in range(2)] for k in range(2)]
            wc = [[consts.tile([128, 128], F16, name=f"wc{k}{mc}", tag=f"wc{k}{mc}")
                   for mc in range(2)] for k in range(2)]
            for k in range(2):
                for mc in range(2):
                    nc.sync.dma_start(wa[k][mc][:], WA[128 * k:128 * (k + 1), 128 * mc:128 * (mc + 1)])
                    nc.sync.dma_start(wc[k][mc][:], WC[128 * k:128 * (k + 1), 128 * mc:128 * (mc + 1)])
            by_sb = consts.tile([128, 2], F32, tag="by")
            nc.sync.dma_start(by_sb[:, 0:1], BY[0:128, :])
            nc.sync.dma_start(by_sb[:, 1:2], BY[128:256, :])
            ident_f32 = consts.tile([128, 128], F32, tag="ident_f32")
            make_identity(nc, ident_f32[:])
            ident = consts.tile([128, 128], F16, tag="ident")
            nc.vector.tensor_copy(ident[:], ident_f32[:])

            m = mpool.tile([128, 2 * BLOC], F32, tag="m")   # master fp32 state
            nc.sync.dma_start(m[:, 0:BLOC], X0T[0:128, :])
            nc.sync.dma_start(m[:, BLOC:2 * BLOC], X0T[128:256, :])
            xq = xqpool.tile([128, 2 * BLOC], F16, tag="xq")  # fp16 copy for PE
            nc.vector.tensor_copy(xq[:], m[:])

            # ---- recurrence: M_i = M_{i-1} + STEP*(A@xq + tanh(C@xq + by)) ----
            GRP = 4  # output steps batched per stage copy/DMA
            pt = None
            pend = []
            for t in range(1, n_steps + 1):
                g = (t - 1) % GRP
                gn = min(GRP, n_steps - (t - 1 - g))  # size of this group
                pv = psv.tile([128, 2 * BLOC], F32, tag="pv")
                pu = psu.tile([128, 2 * BLOC], F32, tag="pu")
                # V = C@xq first (feeds the tanh -> chain), then U = A@xq
                for mc in range(2):
                    ms = slice(BLOC * mc, BLOC * (mc + 1))
                    nc.tensor.matmul(pv[:, ms], wc[0][mc][:], xq[:, 0:BLOC],
                                     start=True, stop=False)
                    nc.tensor.matmul(pv[:, ms], wc[1][mc][:], xq[:, BLOC:2 * BLOC],
                                     start=False, stop=True)
                for mc in range(2):
                    ms = slice(BLOC * mc, BLOC * (mc + 1))
                    nc.tensor.matmul(pu[:, ms], wa[0][mc][:], xq[:, 0:BLOC],
                                     start=True, stop=False)
                    nc.tensor.matmul(pu[:, ms], wa[1][mc][:], xq[:, BLOC:2 * BLOC],
                                     start=False, stop=True)

                # P = M + STEP*U  (off the tanh chain; runs while ACT computes tanh)
                p = ppool.tile([128, 2 * BLOC], F32, tag="p")
                nc.vector.scalar_tensor_tensor(
                    p[:], pu[:], STEP, m[:], op0=ALU.mult, op1=ALU.add)

                # tanh per m-chunk (fused per-partition bias), staggered so the
                # next-step k0 matmuls can start as soon as xq chunk0 lands
                tt = tpool.tile([128, 2 * BLOC], F32, tag="tt")
                nc.scalar.activation(tt[:, 0:BLOC], pv[:, 0:BLOC], AF.Tanh,
                                     bias=by_sb[:, 0:1], scale=1.0)
                nc.scalar.activation(tt[:, BLOC:2 * BLOC], pv[:, BLOC:2 * BLOC],
                                     AF.Tanh, bias=by_sb[:, 1:2], scale=1.0)

                # chain ops: next PE input (fp16), per chunk
                xq = xqpool.tile([128, 2 * BLOC], F16, tag="xq")
                nc.vector.scalar_tensor_tensor(
                    xq[:, 0:BLOC], tt[:, 0:BLOC], STEP, p[:, 0:BLOC],
                    op0=ALU.mult, op1=ALU.add)
                nc.vector.scalar_tensor_tensor(
                    xq[:, BLOC:2 * BLOC], tt[:, BLOC:2 * BLOC], STEP,
                    p[:, BLOC:2 * BLOC], op0=ALU.mult, op1=ALU.add)
                # master state, same math in fp32 (off chain)
                m = mpool.tile([128, 2 * BLOC], F32, tag="m")
                nc.vector.scalar_tensor_tensor(
                    m[:], tt[:], STEP, p[:], op0=ALU.mult, op1=ALU.add)

                # output row t: transpose state copy [128, 2b] -> [b, 256]
                # into a GRP-step PSUM batch; emitted one iteration late so the
                # next step's V matmuls outrank the transposes when xq lands
                pend.append((t, g, gn, xq))
                if len(pend) == 2 or t == n_steps:
                    for (tp, gp, gnp, xqp) in (pend if t == n_steps else pend[:1]):
                        if gp == 0:
                            pt = pst.tile([BLOC, GRP, N], F16, tag="pt", name="pt")
                        nc.tensor.transpose(pt[:, gp, 0:128], xqp[:, 0:BLOC], ident[:])
                        nc.tensor.transpose(pt[:, gp, 128:256], xqp[:, BLOC:2 * BLOC], ident[:])
                        if gp == gnp - 1:
                            stage = stpool.tile([BLOC, GRP, N], F32, tag="stage", name="stage")
                            half = (gnp + 1) // 2
                            nc.vector.tensor_copy(stage[:, 0:half], pt[:, 0:half])
                            nc.scalar.copy(stage[:, half:gnp], pt[:, half:gnp])
                            nc.sync.dma_start(OUT[:, tp - gnp + 1:tp + 1, :], stage[:, 0:gnp])
                    pend = pend[1:] if t != n_steps else []
    nc.compile()
    return nc


def kernel(X0, MA, MW, bA_z, bW_z, by_w):
    global LAST_RESULT
    from concourse.bass_utils import run_bass_kernel_spmd

    X0 = np.asarray(X0, dtype=np.float32)
    MA = np.asarray(MA, dtype=np.float32)
    MW = np.asarray(MW, dtype=np.float32)
    bA_z = np.asarray(bA_z, dtype=np.float32)
    bW_z = np.asarray(bW_z, dtype=np.float32)
    by_w = np.asarray(by_w, dtype=np.float32)

    # host-side weight prep (f32, matches reference math); weights to fp16
    # for full-rate PE matmuls (master state stays fp32 on device).
    bA = np.float32(0.5) * np.exp(-bA_z[0, 0] * bA_z[0, 0]) + np.float32(0.5)
    bW = np.float32(0.5) * np.exp(-bW_z[0, 0] * bW_z[0, 0]) + np.float32(0.5)
    I = np.eye(N, dtype=np.float32)
    A = (1 - bA) * (MA + MA.T) + bA * (MA - MA.T) - np.float32(YA) * I
    C = (1 - bA) * (MW + MW.T) + bW * (MW - MW.T) - np.float32(YW) * I
    WA = np.ascontiguousarray(A.T).astype(np.float16)
    WC = np.ascontiguousarray(C.T).astype(np.float16)

    n_steps = TMAX - 1
    in_maps = []
    for i in range(NCORES):
        in_maps.append({
            "WA": WA,
            "WC": WC,
            "BY": by_w,
            "X0T": np.ascontiguousarray(X0[i * BLOC:(i + 1) * BLOC, :].T),
        })

    nc = _build(n_steps)
    res = run_bass_kernel_spmd(nc, in_maps, core_ids=list(range(NCORES)))
    LAST_RESULT = res

    out = np.concatenate([r["OUT"] for r in res.results], axis=0)
    out[:, 0, :] = X0
    return out


if __name__ == "__main__":
    rng = np.random.default_rng(0)
    inputs = {
        "X0": rng.standard_normal((BS, N), dtype=np.float32),
        "MA": rng.standard_normal((N, N), dtype=np.float32) / 16,
        "MW": rng.standard_normal((N, N), dtype=np.float32) / 16,
        "bA_z": np.full((1, 1), 0.65, dtype=np.float32),
        "bW_z": np.full((1, 1), 0.65, dtype=np.float32),
        "by_w": rng.standard_normal((N, 1), dtype=np.float32) / 100,
    }
    out = kernel(**inputs)
    print("out", out.shape, out.dtype, np.abs(out).max())
